# revision 20
# baseline (speedup 1.0000x reference)
"""Multi-head cross-attention Trainium2 kernel.

Full-input contract: kernel(**inputs) takes the complete tensors and returns
the complete output. Internally shards over 8 NeuronCores as
(batch x head-group): core c handles batch c//4 and heads [4*(c%4), 4*(c%4)+4).
Each core computes its partial output  ctx_g @ Wo_g  for its batch; the host
sums the 4 head-group partials per batch and adds bo.

Masked keys (key_mask == 0) contribute exactly zero probability, so the host
compacts key/value to the unmasked rows (padded up to a multiple of 1024 with
-1e9 score bias), which shrinks the K/V projections and the whole attention
core proportionally. The device program is built for the padded key count and
cached per count.

Per-core device pipeline (all matmuls in fp32r):
  qT = (Wq_g^T @ query_b^T)          [256, 1024]   (activations pre-transposed on host)
  kT = (Wk_g^T @ key_b^T)            [256, SKP]
  vT = (Wv_g^T @ value_b^T)          [256, SKP]  -> PE-transpose -> v [SKP, 256]
  per head h: S^T[sk,sq] = kT_h slices (stationary) x qT_h; exp via ACT with
  fused scale + per-partition mask bias; ctx^T and the softmax denominator
  (via a ones column appended to v) accumulate in PSUM over sk; normalize;
  out = ctxT^T @ Wo_g.
"""

import numpy as np

B, SQ, SK, IN = 2, 1024, 4096, 1024
H_TOT, D, HPC = 16, 64, 4
DH = HPC * D  # 256, per-core head-dim slice
NCORES = 8

_CACHE = {}

import os as _os
CFG = {
    "pp": int(_os.environ.get("CFG_PP", "3")),
    "pa": int(_os.environ.get("CFG_PA", "1")),
    "sin": int(_os.environ.get("CFG_SIN", "8")),
    "sexp": int(_os.environ.get("CFG_SEXP", "6")),
    "p1heads": int(_os.environ.get("CFG_P1H", "1")),
    "projb": int(_os.environ.get("CFG_PROJB", "2")),
}


def _build(skp):
    import concourse.tile as tile
    from concourse import bacc, mybir

    FP = mybir.dt.float32
    FR = mybir.dt.float32r
    AF = mybir.ActivationFunctionType
    MUL = mybir.AluOpType.mult

    nc = bacc.Bacc("TRN2", target_bir_lowering=False, debug=False)

    qT_d = nc.dram_tensor("qT", [IN, SQ], FR, kind="ExternalInput").ap()
    kT_d = nc.dram_tensor("kT", [IN, skp], FR, kind="ExternalInput").ap()
    vT_d = nc.dram_tensor("vT", [IN, skp], FR, kind="ExternalInput").ap()
    wq_d = nc.dram_tensor("wq", [IN, DH], FR, kind="ExternalInput").ap()
    wk_d = nc.dram_tensor("wk", [IN, DH], FR, kind="ExternalInput").ap()
    wv_d = nc.dram_tensor("wv", [IN, DH], FR, kind="ExternalInput").ap()
    wo_d = nc.dram_tensor("wo", [DH, SQ], FR, kind="ExternalInput").ap()
    bqkv_d = nc.dram_tensor("bqkv", [128, 6], FP, kind="ExternalInput").ap()
    mb_d = nc.dram_tensor("mb", [128, skp // 128], FP, kind="ExternalInput").ap()
    ones_d = nc.dram_tensor("ones", [1, 128], FR, kind="ExternalInput").ap()
    idn_d = nc.dram_tensor("idn", [128, 128], FR, kind="ExternalInput").ap()
    out_d = nc.dram_tensor("out", [SQ, SQ], FP, kind="ExternalOutput").ap()

    NSKT = skp // 128          # sk tiles of 128
    NKC = IN // 128            # 8 contraction chunks
    SCALE = 1.0 / float(np.sqrt(D))

    with tile.TileContext(nc) as tc:
        # ---- resident tensors (one bufs=1 pool; distinct names = own slots) ----
        cpool_cm = tc.tile_pool(name="const", bufs=1)
        cpool = cpool_cm.__enter__()
        wq_sb = cpool.tile([128, NKC, DH], FR, name="wq_sb")
        wk_sb = cpool.tile([128, NKC, DH], FR, name="wk_sb")
        wv_sb = cpool.tile([128, NKC, DH], FR, name="wv_sb")
        wo_sb = cpool.tile([128, 2, SQ], FR, name="wo_sb")
        bqkv_sb = cpool.tile([128, 6], FP, name="bqkv_sb")
        mb_sb = cpool.tile([128, NSKT], FP, name="mb_sb")
        ones_sb = cpool.tile([1, 128], FR, name="ones_sb")
        idn_sb = cpool.tile([128, 128], FR, name="idn_sb")
        qT_sb = cpool.tile([128, 2, SQ], FR, name="qT_sb")
        kT_sb = cpool.tile([128, 2, skp], FR, name="kT_sb")
        vT_sb = cpool.tile([128, 2, skp], FR, name="vT_sb")
        vext_sb = cpool.tile([128, NSKT, 65 * HPC], FR, name="vext_sb")
        ctxT_sb = cpool.tile([128, 2, SQ], FR, name="ctxT_sb")

        nc.sync.dma_start(out=wq_sb[:], in_=wq_d.rearrange("(kc p) n -> p kc n", p=128))
        nc.sync.dma_start(out=bqkv_sb[:], in_=bqkv_d[:, :])
        nc.sync.dma_start(out=wk_sb[:], in_=wk_d.rearrange("(kc p) n -> p kc n", p=128))
        nc.sync.dma_start(out=wv_sb[:], in_=wv_d.rearrange("(kc p) n -> p kc n", p=128))
        nc.sync.dma_start(out=mb_sb[:], in_=mb_d[:, :])
        nc.sync.dma_start(out=idn_sb[:], in_=idn_d[:, :])
        nc.sync.dma_start(out=ones_sb[:], in_=ones_d[:, :])
        nc.sync.dma_start(out=wo_sb[:], in_=wo_d.rearrange("(t p) n -> p t n", p=128))

        with tc.tile_pool(name="sin", bufs=CFG["sin"]) as sin, \
             tc.tile_pool(name="sexp", bufs=CFG["sexp"]) as sexp, \
             tc.tile_pool(name="sout", bufs=2) as sout, \
             tc.tile_pool(name="pp", bufs=CFG["pp"], space="PSUM") as pp, \
             tc.tile_pool(name="pa", bufs=CFG["pa"], space="PSUM") as pa:

            # ---- projections: out^T = W^T @ x^T, weight-stationary ----
            def blocks_of(width):
                out, off = [], 0
                while off < width:
                    w = min(1024, width - off)
                    out.append((off, w))
                    off += w
                return out

            def proj_block(w_sb, x_d, dst_sb, bias_col0, off, w, eng=None):
                eng = eng or nc.sync
                ps = [pp.tile([128, 1024], FP, tag="mm", name=f"ps{t}")
                      for t in range(2)]
                for kc in range(NKC):
                    xin = sin.tile([128, 1024], FR, tag="sin", name="xin")
                    eng.dma_start(
                        out=xin[:, 0:w],
                        in_=x_d[kc * 128:(kc + 1) * 128, off:off + w])
                    for t in range(2):
                        for lo in range(0, w, 512):
                            nw = min(512, w - lo)
                            nc.tensor.matmul(
                                ps[t][:, lo:lo + nw],
                                lhsT=w_sb[:, kc, t * 128:(t + 1) * 128],
                                rhs=xin[:, lo:lo + nw],
                                start=(kc == 0), stop=(kc == NKC - 1))
                for t in range(2):
                    with nc.allow_low_precision(reason="float32r storage"):
                        nc.vector.tensor_scalar_add(
                            dst_sb[:, t, off:off + w], ps[t][:, 0:w],
                            bqkv_sb[:, bias_col0 + t:bias_col0 + t + 1])

            vv = vext_sb[:, :, :].rearrange("p s (h c) -> p s h c", c=65)

            def transpose_block(t, off, w):
                for skt in range(off // 128, (off + w) // 128):
                    ps_t = pp.tile([128, 128], FR, tag="mm", name="ps_t")
                    nc.tensor.transpose(
                        ps_t[:, :], in_=vT_sb[:, t, skt * 128:(skt + 1) * 128],
                        identity=idn_sb[:, :])
                    with nc.allow_low_precision(reason="float32r storage"):
                        nc.vector.tensor_copy(
                            vv[:, skt, 2 * t:2 * t + 2, 0:64],
                            ps_t[:, :].rearrange("p (h c) -> p h c", c=64))

            def attn_skt(h, acc, skt):
                t, r0 = h // 2, 64 * (h % 2)
                ps_s = pp.tile([128, SQ], FP, tag="mm", name="ps_s")
                for lo in range(0, SQ, 512):
                    nc.tensor.matmul(
                        ps_s[:, lo:lo + 512],
                        lhsT=kT_sb[r0:r0 + 64, t, skt * 128:(skt + 1) * 128],
                        rhs=qT_sb[r0:r0 + 64, t, lo:lo + 512],
                        start=True, stop=True)
                es = sexp.tile([128, SQ], FR, tag="es", name="es")
                nc.scalar.activation(
                    es[:, :], ps_s[:, :], AF.Exp,
                    bias=mb_sb[:, skt:skt + 1], scale=SCALE)
                for lo in range(0, SQ, 512):
                    nc.tensor.matmul(
                        acc[:, lo:lo + 512],
                        lhsT=vext_sb[:, skt, 65 * h:65 * h + 65],
                        rhs=es[:, lo:lo + 512],
                        start=(skt == 0), stop=(skt == NSKT - 1))

            def normalize(h, acc):
                t, r0 = h // 2, 64 * (h % 2)
                rec = sout.tile([1, SQ], FR, tag="rec", name="rec")
                with nc.allow_low_precision(reason="float32r storage"):
                    for lo in range(0, SQ, 512):
                        nc.vector.reciprocal(rec[:, lo:lo + 512],
                                             acc[64:65, lo:lo + 512])
                ps_bc = pp.tile([64, SQ], FP, tag="mm", name="ps_bc")
                for lo in range(0, SQ, 512):
                    nc.tensor.matmul(ps_bc[:, lo:lo + 512],
                                     lhsT=ones_sb[0:1, 0:64],
                                     rhs=rec[0:1, lo:lo + 512],
                                     start=True, stop=True)
                bc_sb = sout.tile([64, SQ], FP, tag="bc", name="bc_sb")
                nc.vector.tensor_copy(bc_sb[:], ps_bc[:])
                with nc.allow_low_precision(reason="float32r storage"):
                    nc.vector.tensor_tensor(ctxT_sb[r0:r0 + 64, t, :],
                                            acc[0:64, :], bc_sb[:, :], MUL)

            # vext ones columns (written once, before any v data lands)
            ones_fp = sout.tile([128, NSKT], FP, tag="onesfp", name="ones_fp")
            nc.vector.memset(ones_fp[:], 1.0)
            with nc.allow_low_precision(reason="float32r has float32 storage"):
                nc.vector.tensor_copy(
                    vv[:, :, :, 64:65],
                    ones_fp[:, :, None, None].to_broadcast((128, NSKT, HPC, 1)))

            # pass 1: Q proj, then per sk-block K/V proj + transposes,
            # interleaved with head-0 attention to keep ACT busy early
            for off, w in blocks_of(SQ):
                proj_block(wq_sb, qT_d, qT_sb, 0, off, w, eng=nc.scalar)
            nh1 = CFG["p1heads"]
            accs = [pa.tile([65, SQ], FP, tag="acc", name="acc")
                    for _ in range(nh1)]
            for off, w in blocks_of(skp):
                proj_block(wk_sb, kT_d, kT_sb, 2, off, w)
                proj_block(wv_sb, vT_d, vT_sb, 4, off, w, eng=nc.scalar)
                transpose_block(0, off, w)
                transpose_block(1, off, w)
                for skt in range(off // 128, (off + w) // 128):
                    for h in range(nh1):
                        attn_skt(h, accs[h], skt)
            for h in range(nh1):
                normalize(h, accs[h])

            # pass 2: remaining heads
            for h in range(nh1, HPC):
                acc = pa.tile([65, SQ], FP, tag="acc", name="acc")
                for skt in range(NSKT):
                    attn_skt(h, acc, skt)
                normalize(h, acc)

            # ---- output projection: out = ctx @ Wo_g ----
            for sq in range(SQ // 128):
                ps_o = pp.tile([128, SQ], FP, tag="mm", name="ps_o")
                for t in range(2):
                    for lo in range(0, SQ, 512):
                        nc.tensor.matmul(
                            ps_o[:, lo:lo + 512],
                            lhsT=ctxT_sb[:, t, sq * 128:(sq + 1) * 128],
                            rhs=wo_sb[:, t, lo:lo + 512],
                            start=(t == 0), stop=(t == 1))
                o_sb = sout.tile([128, SQ], FP, tag="o", name="o_sb")
                nc.vector.tensor_copy(o_sb[:], ps_o[:])
                nc.sync.dma_start(out=out_d[sq * 128:(sq + 1) * 128, :], in_=o_sb[:])

        cpool_cm.__exit__(None, None, None)

    nc.compile()
    return nc


def get_nc(skp=SK):
    key = ("nc", skp)
    if key not in _CACHE:
        _CACHE[key] = _build(skp)
    return _CACHE[key]


def make_in_maps(query, key, value, key_mask, Wq, bq, Wk, bk, Wv, bv, Wo, bo):
    f32 = lambda x: np.asarray(x, dtype=np.float32)
    query, key, value = f32(query), f32(key), f32(value)
    Wq, bq, Wk, bk = f32(Wq), f32(bq), f32(Wk), f32(bk)
    Wv, bv, Wo = f32(Wv), f32(bv), f32(Wo)
    key_mask = np.asarray(key_mask)

    # compact unmasked keys; pad to a common multiple of 128
    keep = [np.nonzero(key_mask[b] != 0)[0] for b in range(B)]
    skp = max(512, int(-(-max(len(k) for k in keep) // 128) * 128))
    skp = min(skp, SK)

    idn = np.eye(128, dtype=np.float32)
    ones = np.ones((1, 128), np.float32)
    qT, kT, vT, mb = [], [], [], []
    for b in range(B):
        n = len(keep[b])
        kc = np.zeros((skp, IN), np.float32)
        vc = np.zeros((skp, IN), np.float32)
        kc[:n] = key[b][keep[b]]
        vc[:n] = value[b][keep[b]]
        mbias = np.full(skp, -1e9, np.float32)
        mbias[:n] = 0.0
        qT.append(np.ascontiguousarray(query[b].T))
        kT.append(np.ascontiguousarray(kc.T))
        vT.append(np.ascontiguousarray(vc.T))
        mb.append(np.ascontiguousarray(mbias.reshape(skp // 128, 128).T))

    in_maps = []
    for c in range(NCORES):
        b, g = c // 4, c % 4
        S = slice(DH * g, DH * (g + 1))
        bqkv = np.stack([bq[S][0:128], bq[S][128:256],
                         bk[S][0:128], bk[S][128:256],
                         bv[S][0:128], bv[S][128:256]], axis=1)
        in_maps.append({
            "qT": qT[b], "kT": kT[b], "vT": vT[b],
            "wq": np.ascontiguousarray(Wq[:, S]),
            "wk": np.ascontiguousarray(Wk[:, S]),
            "wv": np.ascontiguousarray(Wv[:, S]),
            "wo": np.ascontiguousarray(Wo[S, :]),
            "bqkv": np.ascontiguousarray(bqkv),
            "mb": mb[b], "ones": ones, "idn": idn,
        })
    return in_maps, skp


def run(in_maps, skp=SK, trace=False):
    from concourse.bass_utils import run_bass_kernel_spmd
    nc = get_nc(skp)
    res = run_bass_kernel_spmd(nc, in_maps, list(range(NCORES)), trace=trace)
    _CACHE["last_results"] = res
    return res


def kernel(query, key, value, key_mask, Wq, bq, Wk, bk, Wv, bv, Wo, bo):
    in_maps, skp = make_in_maps(query, key, value, key_mask,
                                Wq, bq, Wk, bk, Wv, bv, Wo, bo)
    res = run(in_maps, skp)
    out = np.zeros((B, SQ, SQ), np.float32)
    for c in range(NCORES):
        out[c // 4] += res.results[c]["out"]
    out += np.asarray(bo, np.float32)[None, None, :]
    return out
